# revision 1
# baseline (speedup 1.0000x reference)
"""Cost-volume (left) kernel for Trainium2, 8 NeuronCores, batch-parallel.

Math: since disp_init is uniform in [0,1), floor(x - disp_init - off) ==
x - off - 1 for every integer off (continuous at d=0), so the bilinear
warp collapses to static shifts:

  cost[g, k, h, x] = d * corr[8-k] + (1-d) * corr[9-k]

where corr[i] (i = 0..9, shift j = i-5) is the group-mean correlation

  corr[i](g, h, x) = (1/8) * sum_{c in g} L[c, h, x] * R[c, h, x + i - 5]

with R zero-padded along x.  Verified exactly equivalent (fp-rounding
level) to the bilinear-warp reference for all d in [0, 1).

Per-core layout (1 batch element / core):
  - chunk = 16 h rows; per chunk 8 "pairs" q (2 rows each: hb=0,1)
  - L/R/prod tiles: partitions = (hb, c) [p = 64*hb + c], free = (q, x)
  - group-reduce over c via TensorE: per 32-partition quad r, two
    accumulating matmuls (K=128, M=32) with block-structured selector
    weights; PSUM partitions = (q, hb, g) [p = 16q + 8hb + g]
  - blend on full 128 partitions; single out tile [128, 9, 256] so the
    store DMA merges (g,k) and fits the 3-dim DMA AP limit.
"""

import numpy as np
from contextlib import ExitStack

import sys

if "/opt/trn_rl_repo" not in sys.path:
    sys.path.insert(0, "/opt/trn_rl_repo")

B, C, H, W = 8, 64, 256, 256
G = 8
NS = 10          # shift indices i = 0..9  <->  j = i - 5
KD = 9           # disparity hypotheses
CH = 16          # h rows per chunk
NCHUNK = H // CH
Q = CH // 2      # row-pairs per chunk
XP = 272         # padded R row width (data at cols [5, 261))
PD = 5           # left pad
RB = W           # column where the R block starts inside a packed row
RW = W + XP      # packed row width (L | R-padded)
HW = H * W


def _sel_np() -> np.ndarray:
    """Selector weights [128, 2, 32]: rows p=(hb,c); block qq of a quad
    maps its row-pair to psum partitions m = 16*qq + 8*hb + g."""
    sel = np.zeros((128, 2, 32), np.float32)
    for p in range(128):
        hb, c = p // 64, p % 64
        for qq in range(2):
            sel[p, qq, 16 * qq + 8 * hb + (c // 8)] = 0.125
    return sel


def _build_nc():
    import concourse.bass as bass
    import concourse.bacc as bacc
    import concourse.tile as tile
    from concourse import mybir

    f32 = mybir.dt.float32
    mult = mybir.AluOpType.mult
    add = mybir.AluOpType.add

    nc = bacc.Bacc("TRN2", target_bir_lowering=False, debug=False)
    # host-packed: [hb, c, chunk, q, 0:W]=L, [.., RB+PD:RB+PD+W]=R (zero pad)
    flr = nc.dram_tensor("featlr", [2, C, NCHUNK, Q, RW], f32,
                         kind="ExternalInput").ap()
    dsp = nc.dram_tensor("disp", [H, W], f32, kind="ExternalInput").ap()
    seld = nc.dram_tensor("sel", [128, 2, 32], f32, kind="ExternalInput").ap()
    outd = nc.dram_tensor("out", [G, KD, H, W], f32, kind="ExternalOutput").ap()

    def bcast(ap2, n):
        # [P, X] view -> [P, n, X] with step-0 middle axis
        return bass.AP(tensor=ap2.tensor, offset=ap2.offset,
                       ap=[ap2.ap[0], [0, n], ap2.ap[1]])

    with tile.TileContext(nc) as tc, ExitStack() as ctx:
        singles = ctx.enter_context(tc.tile_pool(name="singles", bufs=1))
        loads = ctx.enter_context(tc.tile_pool(name="loads", bufs=3))
        prods = ctx.enter_context(tc.tile_pool(name="prods", bufs=4))
        psums = ctx.enter_context(tc.tile_pool(name="psums", bufs=2, space="PSUM"))
        tmps = ctx.enter_context(tc.tile_pool(name="tmps", bufs=2))
        outs = ctx.enter_context(tc.tile_pool(name="outs", bufs=2))

        St = singles.tile([128, 2, 32], f32)
        nc.sync.dma_start(out=St, in_=seld)

        for t in range(NCHUNK):
            h0 = t * CH

            LRt = loads.tile([128, Q, RW], f32, tag="LR")
            Dt = loads.tile([128, W], f32, tag="D")

            # ONE contiguous DMA for L+R: partitions (hb,c), free (q, col)
            nc.sync.dma_start(
                out=LRt,
                in_=bass.AP(tensor=flr.tensor, offset=t * Q * RW,
                            ap=[[NCHUNK * Q * RW, 128], [1, Q * RW]]))

            # disp rows replicated across g: partitions (h'=2q+hb, g)
            nc.sync.dma_start(
                out=Dt,
                in_=bass.AP(tensor=dsp.tensor, offset=h0 * W,
                            ap=[[W, CH], [0, G], [1, W]]))

            # products: per row-pair q, all 10 shifts in one op
            ptiles = []
            for q in range(Q):
                pq = prods.tile([128, NS, W], f32, tag="prod")
                base = LRt[:, q, 0:W]
                in0 = bass.AP(tensor=base.tensor, offset=base.offset,
                              ap=[base.ap[0], [0, NS], base.ap[1]])
                rb = LRt[:, q, RB:RB + W]
                in1 = bass.AP(tensor=rb.tensor, offset=rb.offset,
                              ap=[rb.ap[0], [1, NS], rb.ap[1]])
                nc.any.tensor_tensor(pq, in0, in1, mult)
                ptiles.append(pq)

            # group-reduce via PE. psA: shifts 4..9 (k=0..4); psB: 0..4 (k=5..8)
            psA = psums.tile([128, 6, W], f32, tag="corr")
            psB = psums.tile([128, 5, W], f32, tag="corr")
            for r in range(Q // 2):
                p0, p1 = ptiles[2 * r], ptiles[2 * r + 1]
                tp = (0, 32 * r)
                oA = psA[32 * r:32 * r + 32]
                oB = psB[32 * r:32 * r + 32]
                for j0, j1 in ((0, 2), (2, 4), (4, 6)):
                    nc.tensor.matmul(oA[:, j0:j1], St[:, 0, :], p0[:, 4 + j0:4 + j1],
                                     start=True, stop=False, tile_position=tp)
                    nc.tensor.matmul(oA[:, j0:j1], St[:, 1, :], p1[:, 4 + j0:4 + j1],
                                     start=False, stop=True, tile_position=tp)
                for j0, j1 in ((0, 2), (2, 4), (4, 5)):
                    nc.tensor.matmul(oB[:, j0:j1], St[:, 0, :], p0[:, j0:j1],
                                     start=True, stop=False, tile_position=tp)
                    nc.tensor.matmul(oB[:, j0:j1], St[:, 1, :], p1[:, j0:j1],
                                     start=False, stop=True, tile_position=tp)

            # blend: cost(k) = d*corr[8-k] + (1-d)*corr[9-k]
            # (each op reads at most ONE PSUM operand - HW constraint)
            out_sb = outs.tile([128, KD, W], f32, tag="osb")
            oap = out_sb[:, 0, :]

            omd = tmps.tile([128, W], f32, tag="omd")   # 1 - d
            nc.any.tensor_scalar(omd, Dt, -1.0, 1.0, mult, add)

            t1A = tmps.tile([128, 5, W], f32, tag="t1")
            nc.any.tensor_tensor(t1A, psA[:, 0:5, :], bcast(Dt[:, :], 5), mult)
            t2A = tmps.tile([128, 5, W], f32, tag="t2")
            nc.any.tensor_tensor(t2A, psA[:, 1:6, :], bcast(omd[:, :], 5), mult)
            revA = bass.AP(tensor=oap.tensor, offset=oap.offset + 4 * W,
                           ap=[oap.ap[0], [-W, 5], [1, W]])
            nc.any.tensor_tensor(revA, t1A, t2A, add)

            t1B = tmps.tile([128, 4, W], f32, tag="t1")
            nc.any.tensor_tensor(t1B, psB[:, 0:4, :], bcast(Dt[:, :], 4), mult)
            t2B = tmps.tile([128, 4, W], f32, tag="t2")
            nc.any.tensor_tensor(t2B, psB[:, 1:5, :], bcast(omd[:, :], 4), mult)
            revB = bass.AP(tensor=oap.tensor, offset=oap.offset + 8 * W,
                           ap=[oap.ap[0], [-W, 4], [1, W]])
            nc.any.tensor_tensor(revB, t1B, t2B, add)

            # store: partitions (h', g) + free (k, x) -> [g, k, h0+h', x]
            dst = bass.AP(tensor=outd.tensor, offset=h0 * W,
                          ap=[[W, CH], [HW, G * KD], [1, W]])
            nc.sync.dma_start(out=dst, in_=out_sb)

    nc.compile()
    return nc


_NC_CACHE = None


def _get_nc():
    global _NC_CACHE
    if _NC_CACHE is None:
        _NC_CACHE = _build_nc()
    return _NC_CACHE


def _install_profile_hook():
    """Make trace=True work in this container: provide the missing
    antenv.axon_hooks module (ctypes NTFF hook) and stub out the
    artifact upload."""
    import types
    import ctypes
    import contextlib

    if "antenv.axon_hooks" not in sys.modules:
        so_path = "/opt/axon/libaxon_pjrt.so"
        lib = ctypes.CDLL(so_path)
        lib.axon_start_nrt_profile.argtypes = [
            ctypes.POINTER(ctypes.c_int64), ctypes.c_size_t]
        lib.axon_start_nrt_profile.restype = ctypes.c_int64
        lib.axon_stop_nrt_profile.argtypes = [ctypes.c_char_p]
        lib.axon_stop_nrt_profile.restype = ctypes.c_int64

        @contextlib.contextmanager
        def _hook(output_dir, device_ids):
            import jax
            jax.devices()
            if device_ids:
                ids = (ctypes.c_int64 * len(device_ids))(*device_ids)
                rc = lib.axon_start_nrt_profile(ids, len(device_ids))
            else:
                rc = lib.axon_start_nrt_profile(None, 0)
            if rc != 0:
                raise RuntimeError(f"axon_start_nrt_profile rc={rc}")
            try:
                yield
            finally:
                n = lib.axon_stop_nrt_profile(str(output_dir).encode())
                print(f"profile: {n} file(s) written to {output_dir}",
                      file=sys.stderr)

        mod = types.ModuleType("antenv.axon_hooks")
        mod._hook = _hook
        mod.get_axon_ntff_profile_hook = lambda: _hook
        mod.set_axon_ntff_profile_hook = lambda h: None
        sys.modules["antenv.axon_hooks"] = mod

    import concourse.bass_utils as bu
    bu.upload_artifacts = lambda tmpdir: f"local:{tmpdir}"


def run(feat_left, feat_right, disp_init, trace=False):
    if trace:
        _install_profile_hook()
    from concourse.bass_utils import run_bass_kernel_spmd

    nc = _get_nc()
    sel = _sel_np()
    fl = np.asarray(feat_left, dtype=np.float32)
    fr = np.asarray(feat_right, dtype=np.float32)
    dd = np.ascontiguousarray(np.asarray(disp_init, dtype=np.float32))

    # [C,H,W] -> [hb, c, chunk, q, x]; pack [L | R-zero-padded] per row
    def _rearr(a):
        return a.reshape(C, NCHUNK, Q, 2, W).transpose(3, 0, 1, 2, 4)

    in_maps = []
    for b in range(B):
        flr = np.zeros((2, C, NCHUNK, Q, RW), np.float32)
        flr[..., 0:W] = _rearr(fl[b])
        flr[..., RB + PD:RB + PD + W] = _rearr(fr[b])
        in_maps.append({
            "featlr": flr,
            "disp": dd[b, 0],
            "sel": sel,
        })
    res = run_bass_kernel_spmd(nc, in_maps, core_ids=list(range(B)), trace=trace)
    out = np.stack([res.results[b]["out"] for b in range(B)], axis=0)
    return out, res


def kernel(feat_left, feat_right, disp_init):
    out, _ = run(feat_left, feat_right, disp_init)
    return out



# revision 2
# speedup vs baseline: 1.5633x; 1.5633x over previous
"""Cost-volume (left) kernel for Trainium2, 8 NeuronCores, batch-parallel.

Math: since disp_init is uniform in [0,1), floor(x - disp_init - off) ==
x - off - 1 for every integer off (continuous at d=0), so the bilinear
warp collapses to static shifts:

  cost[g, k, h, x] = corr[9-k] + d * (corr[8-k] - corr[9-k])

where corr[i] (i = 0..9, shift j = i-5) is the group-mean correlation

  corr[i](g, h, x) = (1/8) * sum_{c in g} L[c, h, x] * R[c, h, x + i - 5]

with R zero-padded along x.

Per-core layout (1 batch element / core), all bf16 on-chip:
  - chunk = 16 h rows; per chunk 4 "quads" v (4 rows each)
  - L/R partitions = (par, hb, g, c4): p = 64*par + 32*hb + 4*g + c4,
    channel = 8*g + 4*s + c4 (s = ctile 0/1), row-in-chunk = 4v+2par+hb
  - R packed TWICE per row at both byte parities so every shifted
    product window is 4B-aligned -> DVE tensor_tensor runs in 2x mode
  - products bf16 [128, 10, 256] per (s, v): DVE (v<3) / GpSimd (v=3)
  - group-reduce on TensorE: ONE fixed selector stationary [128, 32]
    (m = 16*par + 8*hb + g) serves all 4 column groups (tile_position
    (0,32v)) -> no weight reloads between matmuls, col-tiled matmuls
    can overlap; bf16 moving operand streams at 1 cycle/row (fp32 is 4)
  - psum partitions = 8*h' + g; shifts 0..5 in psA, 5..9 in psB
  - blend: ACT copies psum->SBUF bf16 (cs); DVE: diff = cs[i]-cs[i+1],
    m = d*diff, out[k] = cs[9-k] + m[8-k] (all 2x-mode bf16)
  - output stored bf16, upconverted to fp32 on host (tol 2e-2 >> 4e-3)
"""

import numpy as np
from contextlib import ExitStack

import sys

if "/opt/trn_rl_repo" not in sys.path:
    sys.path.insert(0, "/opt/trn_rl_repo")

import ml_dtypes

BF16 = ml_dtypes.bfloat16

B, C, H, W = 8, 64, 256, 256
G = 8
NS = 10          # shift indices i = 0..9  <->  j = i - 5
KD = 9           # disparity hypotheses
CH = 16          # h rows per chunk
NCHUNK = H // CH
NV = 4           # quads per chunk (4 rows each)
ROWLEN = 800     # L(256) | Rpad0(272) | Rpad1(272)
R0 = 256         # Rpad0 block start; R data at R0+5 (even parity windows)
R1 = 528         # Rpad1 block start; R data at R1+4 (odd parity windows)
HW = H * W


def _sel_np() -> np.ndarray:
    """Selector [128, 32]: row p=(par,hb,g,c4) -> col m = 16*par+8*hb+g."""
    sel = np.zeros((128, 32), np.float32)
    for p in range(128):
        par, hb, g = p // 64, (p // 32) % 2, (p % 32) // 4
        sel[p, 16 * par + 8 * hb + g] = 0.125
    return sel.astype(BF16)


def _build_nc():
    import concourse.bass as bass
    import concourse.bacc as bacc
    import concourse.tile as tile
    from concourse import mybir

    f32 = mybir.dt.float32
    bf16 = mybir.dt.bfloat16
    mult = mybir.AluOpType.mult
    add = mybir.AluOpType.add
    sub = mybir.AluOpType.subtract

    nc = bacc.Bacc("TRN2", target_bir_lowering=False, debug=False)
    flr = nc.dram_tensor("featlr", [2, 128, NCHUNK, NV, ROWLEN], bf16,
                         kind="ExternalInput").ap()
    dsp = nc.dram_tensor("disp", [H, W], bf16, kind="ExternalInput").ap()
    seld = nc.dram_tensor("sel", [128, 32], bf16, kind="ExternalInput").ap()
    outd = nc.dram_tensor("out", [G, KD, H, W], bf16, kind="ExternalOutput").ap()

    def bcast(ap2, n):
        # [P, X] view -> [P, n, X] with step-0 middle axis
        return bass.AP(tensor=ap2.tensor, offset=ap2.offset,
                       ap=[ap2.ap[0], [0, n], ap2.ap[1]])

    def rev(t, a, n):
        # reversed middle-axis view of tile t slots [a, a+n) -> a+n-1 .. a
        return bass.AP(tensor=t.tensor, offset=t.offset + (a + n - 1) * W,
                       ap=[t.ap[0], [-W, n], [1, W]])

    with tile.TileContext(nc) as tc, ExitStack() as ctx:
        singles = ctx.enter_context(tc.tile_pool(name="singles", bufs=1))
        loads = ctx.enter_context(tc.tile_pool(name="loads", bufs=2))
        prods = ctx.enter_context(tc.tile_pool(name="prods", bufs=12))
        psums = ctx.enter_context(tc.tile_pool(name="psums", bufs=2, space="PSUM"))
        css = ctx.enter_context(tc.tile_pool(name="css", bufs=2))
        tmps = ctx.enter_context(tc.tile_pool(name="tmps", bufs=2))
        outs = ctx.enter_context(tc.tile_pool(name="outs", bufs=2))

        St = singles.tile([128, 32], bf16)
        nc.sync.dma_start(out=St, in_=seld)

        for t in range(NCHUNK):
            h0 = t * CH

            LR = []
            for s in range(2):
                lrt = loads.tile([128, NV, ROWLEN], bf16, tag=f"LR{s}",
                                 name=f"LR{s}_{t}")
                nc.sync.dma_start(
                    out=lrt,
                    in_=bass.AP(tensor=flr.tensor,
                                offset=s * 128 * NCHUNK * NV * ROWLEN
                                + t * NV * ROWLEN,
                                ap=[[NCHUNK * NV * ROWLEN, 128],
                                    [1, NV * ROWLEN]]))
                LR.append(lrt)

            # disp rows replicated across g: partitions (h', g) = 8h'+g
            Dt = loads.tile([128, W], bf16, tag="D")
            nc.sync.dma_start(
                out=Dt,
                in_=bass.AP(tensor=dsp.tensor, offset=h0 * W,
                            ap=[[W, CH], [0, G], [1, W]]))

            # products: per (ctile s, quad v), 10 shifts in 2 ops
            # (even-parity windows from Rpad0, odd from Rpad1 -> 2x mode)
            X = [[None] * NV for _ in range(2)]
            for s in range(2):
                lrt = LR[s]
                for v in range(NV):
                    pq = prods.tile([128, NS, W], bf16, tag="prod",
                                    name=f"prod_{t}_{s}_{v}")
                    eng = nc.gpsimd if v == 3 else nc.vector
                    l_ap = bass.AP(tensor=lrt.tensor,
                                   offset=lrt.offset + v * ROWLEN,
                                   ap=[lrt.ap[0], [0, 5], [1, W]])
                    for par, rbase in ((0, R0), (1, R1)):
                        out_ap = bass.AP(tensor=pq.tensor,
                                         offset=pq.offset + par * W,
                                         ap=[pq.ap[0], [2 * W, 5], [1, W]])
                        r_ap = bass.AP(tensor=lrt.tensor,
                                       offset=lrt.offset + v * ROWLEN + rbase,
                                       ap=[lrt.ap[0], [2, 5], [1, W]])
                        eng.tensor_tensor(out_ap, l_ap, r_ap, mult)
                    X[s][v] = pq

            # group-reduce via PE: fixed stationary, col-tiled across quads
            psA = psums.tile([128, 6, W], f32, tag="corr")   # shifts 0..5
            psB = psums.tile([128, 5, W], f32, tag="corr")   # shifts 5..9
            groups = [(psA, 0, 0, 2), (psA, 0, 2, 4), (psA, 0, 4, 6),
                      (psB, 5, 0, 2), (psB, 5, 2, 4), (psB, 5, 4, 5)]
            for ps, base, j0, j1 in groups:
                for s in range(2):
                    for v in range(NV):
                        rhs = X[s][v][:, base + j0:base + j1, :]
                        nc.tensor.matmul(ps[32 * v:32 * v + 32, j0:j1],
                                         St[:, :], rhs,
                                         start=(s == 0), stop=(s == 1),
                                         tile_position=(0, 32 * v))

            # blend: cost[k] = cs[9-k] + d * (cs[8-k] - cs[9-k])
            csA = css.tile([128, 6, W], bf16, tag="csA")
            csB = css.tile([128, 5, W], bf16, tag="csB")
            nc.scalar.copy(csA, psA)
            nc.scalar.copy(csB, psB)

            diff = tmps.tile([128, KD, W], bf16, tag="diff")
            nc.vector.tensor_tensor(diff[:, 0:5], csA[:, 0:5], csA[:, 1:6], sub)
            nc.vector.tensor_tensor(diff[:, 5:9], csB[:, 0:4], csB[:, 1:5], sub)

            md = tmps.tile([128, KD, W], bf16, tag="md")
            nc.vector.tensor_tensor(md, diff, bcast(Dt[:, :], KD), mult)

            out_sb = outs.tile([128, KD, W], bf16, tag="osb")
            nc.vector.tensor_tensor(out_sb[:, 0:5], rev(csB, 0, 5),
                                    rev(md, 4, 5), add)
            nc.vector.tensor_tensor(out_sb[:, 5:9], rev(csA, 1, 4),
                                    rev(md, 0, 4), add)

            # store: partitions (h', g) + free (k, x) -> [g, k, h0+h', x]
            dst = bass.AP(tensor=outd.tensor, offset=h0 * W,
                          ap=[[W, CH], [HW, G * KD], [1, W]])
            nc.sync.dma_start(out=dst, in_=out_sb)

    nc.compile()
    return nc


_NC_CACHE = None


def _get_nc():
    global _NC_CACHE
    if _NC_CACHE is None:
        _NC_CACHE = _build_nc()
    return _NC_CACHE


def _install_profile_hook():
    """Make trace=True work in this container: provide the missing
    antenv.axon_hooks module (ctypes NTFF hook) and stub out the
    artifact upload."""
    import types
    import ctypes
    import contextlib

    if "antenv.axon_hooks" not in sys.modules:
        so_path = "/opt/axon/libaxon_pjrt.so"
        lib = ctypes.CDLL(so_path)
        lib.axon_start_nrt_profile.argtypes = [
            ctypes.POINTER(ctypes.c_int64), ctypes.c_size_t]
        lib.axon_start_nrt_profile.restype = ctypes.c_int64
        lib.axon_stop_nrt_profile.argtypes = [ctypes.c_char_p]
        lib.axon_stop_nrt_profile.restype = ctypes.c_int64

        @contextlib.contextmanager
        def _hook(output_dir, device_ids):
            import jax
            jax.devices()
            if device_ids:
                ids = (ctypes.c_int64 * len(device_ids))(*device_ids)
                rc = lib.axon_start_nrt_profile(ids, len(device_ids))
            else:
                rc = lib.axon_start_nrt_profile(None, 0)
            if rc != 0:
                raise RuntimeError(f"axon_start_nrt_profile rc={rc}")
            try:
                yield
            finally:
                n = lib.axon_stop_nrt_profile(str(output_dir).encode())
                print(f"profile: {n} file(s) written to {output_dir}",
                      file=sys.stderr)

        mod = types.ModuleType("antenv.axon_hooks")
        mod._hook = _hook
        mod.get_axon_ntff_profile_hook = lambda: _hook
        mod.set_axon_ntff_profile_hook = lambda h: None
        sys.modules["antenv.axon_hooks"] = mod

    import concourse.bass_utils as bu
    bu.upload_artifacts = lambda tmpdir: f"local:{tmpdir}"


def _pack_lr(fl_b, fr_b):
    """[C,H,W] fp32 x2 -> featlr [2, 128, NCHUNK, NV, ROWLEN] bf16."""
    def rearr(a):
        # C = (g 8, s 2, c4 4); H = (t 16, v 4, par 2, hb 2)
        x = a.reshape(G, 2, 4, NCHUNK, NV, 2, 2, W)
        # -> (s, par, hb, g, c4, t, v, x)
        return x.transpose(1, 5, 6, 0, 2, 3, 4, 7).reshape(
            2, 128, NCHUNK, NV, W)

    arr = np.zeros((2, 128, NCHUNK, NV, ROWLEN), BF16)
    arr[..., 0:W] = rearr(fl_b)
    r = rearr(fr_b).astype(BF16)
    arr[..., R0 + 5:R0 + 5 + W] = r
    arr[..., R1 + 4:R1 + 4 + W] = r
    return arr


def run(feat_left, feat_right, disp_init, trace=False):
    if trace:
        _install_profile_hook()
    from concourse.bass_utils import run_bass_kernel_spmd

    nc = _get_nc()
    sel = _sel_np()
    fl = np.asarray(feat_left, dtype=np.float32)
    fr = np.asarray(feat_right, dtype=np.float32)
    dd = np.ascontiguousarray(np.asarray(disp_init, dtype=np.float32))

    in_maps = []
    for b in range(B):
        in_maps.append({
            "featlr": _pack_lr(fl[b], fr[b]),
            "disp": dd[b, 0].astype(BF16),
            "sel": sel,
        })
    res = run_bass_kernel_spmd(nc, in_maps, core_ids=list(range(B)), trace=trace)
    out = np.stack([np.asarray(res.results[b]["out"]).astype(np.float32)
                    for b in range(B)], axis=0)
    return out, res


def kernel(feat_left, feat_right, disp_init):
    out, _ = run(feat_left, feat_right, disp_init)
    return out


# revision 6
# speedup vs baseline: 2.2517x; 1.4403x over previous
"""Cost-volume (left) kernel for Trainium2, 8 NeuronCores, batch-parallel.

Math: since disp_init is uniform in [0,1), floor(x - disp_init - off) ==
x - off - 1 for every integer off (continuous at d=0), so the bilinear
warp collapses to static shifts:

  cost[g, k, h, x] = corr[9-k] + d * (corr[8-k] - corr[9-k])

where corr[i] (i = 0..9, shift j = i-5) is the group-mean correlation

  corr[i](g, h, x) = (1/8) * sum_{c in g} L[c, h, x] * R[c, h, x + i - 5]

with R zero-padded along x.

Per-core layout (1 batch element / core), all bf16 on-chip:
  - chunk = 16 h rows; per chunk 4 "quads" v (4 rows each)
  - L/R partitions = (par, hb, g, c4): p = 64*par + 32*hb + 4*g + c4,
    channel = 8*g + 4*s + c4 (s = ctile 0/1), row-in-chunk = 4v+2par+hb
  - R packed TWICE per row at both byte parities so every shifted
    product window is 4B-aligned -> DVE tensor_tensor runs in 2x mode
  - products bf16 [128, 10, 256] per (s, v): DVE (v<3) / GpSimd (v=3)
  - group-reduce on TensorE: ONE fixed selector stationary [128, 32]
    (m = 16*par + 8*hb + g) serves all 4 column groups (tile_position
    (0,32v)) -> no weight reloads between matmuls, col-tiled matmuls
    can overlap; bf16 moving operand streams at 1 cycle/row (fp32 is 4)
  - psum partitions = 8*h' + g; shifts 0..5 in psA, 5..9 in psB
  - blend: ACT copies psum->SBUF bf16 (cs); DVE: diff = cs[i]-cs[i+1],
    m = d*diff, out[k] = cs[9-k] + m[8-k] (all 2x-mode bf16)
  - output stored bf16, upconverted to fp32 on host (tol 2e-2 >> 4e-3)
"""

import numpy as np
from contextlib import ExitStack

import sys

if "/opt/trn_rl_repo" not in sys.path:
    sys.path.insert(0, "/opt/trn_rl_repo")

import ml_dtypes

BF16 = ml_dtypes.bfloat16

B, C, H, W = 8, 64, 256, 256
G = 8
NS = 10          # shift indices i = 0..9  <->  j = i - 5
KD = 9           # disparity hypotheses
CH = 16          # h rows per chunk
NCHUNK = H // CH
NV = 4           # quads per chunk (4 rows each)
ROWLEN = 800     # L(256) | Rpad0(272) | Rpad1(272)
R0 = 256         # Rpad0 block start; R data at R0+5 (even parity windows)
R1 = 528         # Rpad1 block start; R data at R1+4 (odd parity windows)
HW = H * W


def _sel_np() -> np.ndarray:
    """Selector [128, 32]: row p=(par,hb,g,c4) -> col m = 16*par+8*hb+g."""
    sel = np.zeros((128, 32), np.float32)
    for p in range(128):
        par, hb, g = p // 64, (p // 32) % 2, (p % 32) // 4
        sel[p, 16 * par + 8 * hb + g] = 0.125
    return sel.astype(BF16)


def _build_nc():
    import concourse.bass as bass
    import concourse.bacc as bacc
    import concourse.tile as tile
    from concourse import mybir

    f32 = mybir.dt.float32
    bf16 = mybir.dt.bfloat16
    mult = mybir.AluOpType.mult
    add = mybir.AluOpType.add
    sub = mybir.AluOpType.subtract

    nc = bacc.Bacc("TRN2", target_bir_lowering=False, debug=False)
    flr = nc.dram_tensor("featlr", [2, 128, NCHUNK, NV, ROWLEN], bf16,
                         kind="ExternalInput").ap()
    dsp = nc.dram_tensor("disp", [H, W], bf16, kind="ExternalInput").ap()
    seld = nc.dram_tensor("sel", [128, 32], bf16, kind="ExternalInput").ap()
    identd = nc.dram_tensor("ident", [128, 128], bf16, kind="ExternalInput").ap()
    outd = nc.dram_tensor("out", [G, KD, H, W], bf16, kind="ExternalOutput").ap()

    def bcast(ap2, n):
        # [P, X] view -> [P, n, X] with step-0 middle axis
        return bass.AP(tensor=ap2.tensor, offset=ap2.offset,
                       ap=[ap2.ap[0], [0, n], ap2.ap[1]])

    def rev(t, a, n):
        # reversed middle-axis view of tile t slots [a, a+n) -> a+n-1 .. a
        return bass.AP(tensor=t.tensor, offset=t.offset + (a + n - 1) * W,
                       ap=[t.ap[0], [-W, n], [1, W]])

    with tile.TileContext(nc) as tc, ExitStack() as ctx:
        singles = ctx.enter_context(tc.tile_pool(name="singles", bufs=1))
        loads = ctx.enter_context(tc.tile_pool(name="loads", bufs=2))
        prods = ctx.enter_context(tc.tile_pool(name="prods", bufs=2))
        psums = ctx.enter_context(tc.tile_pool(name="psums", bufs=2, space="PSUM"))
        css = ctx.enter_context(tc.tile_pool(name="css", bufs=2))
        tmps = ctx.enter_context(tc.tile_pool(name="tmps", bufs=2))
        outs = ctx.enter_context(tc.tile_pool(name="outs", bufs=2))

        St = singles.tile([128, 32], bf16)
        nc.sync.dma_start(out=St, in_=seld)
        It = singles.tile([128, 128], bf16)
        nc.sync.dma_start(out=It, in_=identd)

        for t in range(NCHUNK):
            h0 = t * CH

            LR = []
            for s in range(2):
                lrt = loads.tile([128, NV, ROWLEN], bf16, tag=f"LR{s}",
                                 name=f"LR{s}_{t}")
                nc.sync.dma_start(
                    out=lrt,
                    in_=bass.AP(tensor=flr.tensor,
                                offset=s * 128 * NCHUNK * NV * ROWLEN
                                + t * NV * ROWLEN,
                                ap=[[NCHUNK * NV * ROWLEN, 128],
                                    [1, NV * ROWLEN]]))
                LR.append(lrt)

            # disp rows replicated across g: partitions (h', g) = 8h'+g
            Dt = loads.tile([128, W], bf16, tag="D")
            nc.sync.dma_start(
                out=Dt,
                in_=bass.AP(tensor=dsp.tensor, offset=h0 * W,
                            ap=[[W, CH], [0, G], [1, W]]))

            # products: per ctile s, one big [128, 4v, 10, 256] tile,
            # 2 DVE ops (even-parity windows from Rpad0, odd from Rpad1),
            # all 4 quads per op -> 2x mode, amortized op overhead
            X = []
            for s in range(2):
                lrt = LR[s]
                pq = prods.tile([128, NV, NS, W], bf16, tag=f"prod{s}",
                                name=f"prod_{t}_{s}")
                for par, rbase in ((0, R0), (1, R1)):
                    out_ap = bass.AP(tensor=pq.tensor,
                                     offset=pq.offset + par * W,
                                     ap=[pq.ap[0], [NS * W, NV],
                                         [2 * W, 5], [1, W]])
                    l_ap = bass.AP(tensor=lrt.tensor, offset=lrt.offset,
                                   ap=[lrt.ap[0], [ROWLEN, NV],
                                       [0, 5], [1, W]])
                    r_ap = bass.AP(tensor=lrt.tensor,
                                   offset=lrt.offset + rbase,
                                   ap=[lrt.ap[0], [ROWLEN, NV],
                                       [2, 5], [1, W]])
                    nc.vector.tensor_tensor(out_ap, l_ap, r_ap, mult)
                X.append(pq)

            # group-reduce via PE: fixed stationary, col-tiled across quads
            psA = psums.tile([128, 6, W], f32, tag="corr")   # shifts 0..5
            psB = psums.tile([128, 5, W], f32, tag="corr")   # shifts 5..9
            groups = [(psA, 0, 0, 2), (psA, 0, 2, 4), (psA, 0, 4, 6),
                      (psB, 5, 0, 2), (psB, 5, 2, 4), (psB, 5, 4, 5)]
            for ps, base, j0, j1 in groups:
                for s in range(2):
                    for v in range(NV):
                        rhs = X[s][:, v, base + j0:base + j1, :]
                        nc.tensor.matmul(ps[32 * v:32 * v + 32, j0:j1],
                                         St[:, :], rhs,
                                         start=(s == 0), stop=(s == 1),
                                         tile_position=(0, 32 * v))

            # blend: cost[k] = cs[9-k] + d * (cs[8-k] - cs[9-k])
            csA = css.tile([128, 6, W], bf16, tag="csA")
            csB = css.tile([128, 5, W], bf16, tag="csB")
            nc.scalar.copy(csA, psA)
            nc.scalar.copy(csB, psB)

            diff = tmps.tile([128, KD, W], bf16, tag="diff")
            nc.vector.tensor_tensor(diff[:, 0:5], csA[:, 0:5], csA[:, 1:6], sub)
            nc.vector.tensor_tensor(diff[:, 5:9], csB[:, 0:4], csB[:, 1:5], sub)

            # md[i] = d * diff[i]
            md = tmps.tile([128, KD, W], bf16, tag="md")
            nc.vector.tensor_tensor(md, diff, bcast(Dt[:, :], KD), mult)

            # accumulate md into psum via identity matmuls (all forward):
            # cost[k] = corr[9-k] + md[8-k], so
            # psB[j] += md[4+j] (j=0..4); psA[j] += md[j-1] (j=1..4)
            # (per-MM output must stay inside one 2KB psum bank)
            for ps, j0, j1, m0 in ((psB, 0, 2, 4), (psB, 2, 4, 6),
                                   (psB, 4, 5, 8), (psA, 1, 2, 0),
                                   (psA, 2, 4, 1), (psA, 4, 5, 3)):
                n = j1 - j0
                nc.tensor.matmul(ps[:, j0:j1], It[:, :],
                                 md[:, m0:m0 + n, :],
                                 start=False, stop=True,
                                 skip_group_check=True)

            # stored slot k' = cost[8-k']; host flips the k axis.
            # k'=0..3 <- psA[1+k'] ; k'=4..8 <- psB[k'-4]
            out_sb = outs.tile([128, KD, W], bf16, tag="osb")
            nc.scalar.copy(out_sb[:, 0:4], psA[:, 1:5])
            nc.scalar.copy(out_sb[:, 4:9], psB[:, 0:5])

            # store: partitions (h', g) + free (k, x) -> [g, k, h0+h', x]
            dst = bass.AP(tensor=outd.tensor, offset=h0 * W,
                          ap=[[W, CH], [HW, G * KD], [1, W]])
            nc.sync.dma_start(out=dst, in_=out_sb)

    nc.compile()
    return nc


_NC_CACHE = None


def _get_nc():
    global _NC_CACHE
    if _NC_CACHE is None:
        _NC_CACHE = _build_nc()
    return _NC_CACHE


def _install_profile_hook():
    """Make trace=True work in this container: provide the missing
    antenv.axon_hooks module (ctypes NTFF hook) and stub out the
    artifact upload."""
    import types
    import ctypes
    import contextlib

    if "antenv.axon_hooks" not in sys.modules:
        so_path = "/opt/axon/libaxon_pjrt.so"
        lib = ctypes.CDLL(so_path)
        lib.axon_start_nrt_profile.argtypes = [
            ctypes.POINTER(ctypes.c_int64), ctypes.c_size_t]
        lib.axon_start_nrt_profile.restype = ctypes.c_int64
        lib.axon_stop_nrt_profile.argtypes = [ctypes.c_char_p]
        lib.axon_stop_nrt_profile.restype = ctypes.c_int64

        @contextlib.contextmanager
        def _hook(output_dir, device_ids):
            import jax
            jax.devices()
            if device_ids:
                ids = (ctypes.c_int64 * len(device_ids))(*device_ids)
                rc = lib.axon_start_nrt_profile(ids, len(device_ids))
            else:
                rc = lib.axon_start_nrt_profile(None, 0)
            if rc != 0:
                raise RuntimeError(f"axon_start_nrt_profile rc={rc}")
            try:
                yield
            finally:
                n = lib.axon_stop_nrt_profile(str(output_dir).encode())
                print(f"profile: {n} file(s) written to {output_dir}",
                      file=sys.stderr)

        mod = types.ModuleType("antenv.axon_hooks")
        mod._hook = _hook
        mod.get_axon_ntff_profile_hook = lambda: _hook
        mod.set_axon_ntff_profile_hook = lambda h: None
        sys.modules["antenv.axon_hooks"] = mod

    import concourse.bass_utils as bu
    bu.upload_artifacts = lambda tmpdir: f"local:{tmpdir}"


def _pack_lr(fl_b, fr_b):
    """[C,H,W] fp32 x2 -> featlr [2, 128, NCHUNK, NV, ROWLEN] bf16."""
    def rearr(a):
        # C = (g 8, s 2, c4 4); H = (t 16, v 4, par 2, hb 2)
        x = a.reshape(G, 2, 4, NCHUNK, NV, 2, 2, W)
        # -> (s, par, hb, g, c4, t, v, x)
        return x.transpose(1, 5, 6, 0, 2, 3, 4, 7).reshape(
            2, 128, NCHUNK, NV, W)

    arr = np.zeros((2, 128, NCHUNK, NV, ROWLEN), BF16)
    arr[..., 0:W] = rearr(fl_b)
    r = rearr(fr_b).astype(BF16)
    arr[..., R0 + 5:R0 + 5 + W] = r
    arr[..., R1 + 4:R1 + 4 + W] = r
    return arr


def run(feat_left, feat_right, disp_init, trace=False):
    if trace:
        _install_profile_hook()
    from concourse.bass_utils import run_bass_kernel_spmd

    nc = _get_nc()
    sel = _sel_np()
    ident = np.eye(128, dtype=np.float32).astype(BF16)
    fl = np.asarray(feat_left, dtype=np.float32)
    fr = np.asarray(feat_right, dtype=np.float32)
    dd = np.ascontiguousarray(np.asarray(disp_init, dtype=np.float32))

    in_maps = []
    for b in range(B):
        in_maps.append({
            "featlr": _pack_lr(fl[b], fr[b]),
            "disp": dd[b, 0].astype(BF16),
            "sel": sel,
            "ident": ident,
        })
    res = run_bass_kernel_spmd(nc, in_maps, core_ids=list(range(B)), trace=trace)
    out = np.stack([np.asarray(res.results[b]["out"])[:, ::-1].astype(
        np.float32) for b in range(B)], axis=0)
    return out, res


def kernel(feat_left, feat_right, disp_init):
    out, _ = run(feat_left, feat_right, disp_init)
    return out
